# revision 1
# baseline (speedup 1.0000x reference)
"""Trainium2 Bass kernel for nn_Block_56427280335230 (dense transformer block).

Reference semantics (B=2, L=2048, H=16, D=64, HID=1024):
    h = LayerNorm(x) * ln_w + ln_b
    h[..., :128] = cumlogsumexp(h[..., :128] * 5, axis=seq) / 5
    qkvp = h @ w_in.T ; split q,k,v,p
    q,k = rope(q), rope(k)
    o = softmax(q k^T / 8 + causal) v
    out = concat([o, gelu(p)]) @ w_out.T + b_out

Sharding: DP2 x TP4 over 8 NeuronCores. Cores 0-3 take batch 0, cores 4-7
batch 1. Within a group of 4, heads (4 per core) and the qkvp/vp weight
columns are sharded. Each core computes a full partial out^T [1024, 2048]
over its vp shard; a ReduceScatter over the 4-core group leaves each core
with a disjoint 256-channel slice of the summed output. The host
concatenates the 8 disjoint shards (pure gather, no host reduction).

On-device dataflow is feature-major (channels on partitions, tokens on the
free axis) end to end, so no activation transposes are needed. LayerNorm is
folded into the QKVP matmul via two augmented contraction rows (-mu and
sqrt(var+eps)) with host-augmented weights; the per-token rstd scale is
applied on the PSUM->SBUF pass. The soft-prefix-max uses the DVE prefix-scan
(exp -> cumsum -> ln). Attention computes S^T blocks (keys on partitions) so
exp(S^T) @ V needs no transposes; the softmax denominator rides as a
ones-column in the AV matmul. All matmuls are bf16 with fp32 accumulation.
"""
import numpy as np
import ml_dtypes
from contextlib import ExitStack

from concourse import bass, mybir, tile, bacc
from concourse.masks import make_identity

F32 = mybir.dt.float32
BF16 = mybir.dt.bfloat16

B, L, H, D = 2, 2048, 16, 64
HID = H * D                  # 1024
ACC = HID // 8               # 128 scan channels
N_CORES = 8
TP = 4                       # tensor-parallel group size
HPC = H // TP                # heads per core = 4
CH = 512                     # tokens per chunk
NCH = L // CH                # 4 chunks
KB = 128                     # key block
NKB = L // KB                # 16 key blocks
KQ = 8                       # qkvp contraction tiles: 7 centered-x + 1 part
MQK, MV, MP = HPC, HPC // 2, 8
MTOT = MQK + MV + MP         # 14 m-tiles of the qkvp output (1792 rows)
MO = 8                       # out-proj m-tiles (1024 out channels)
KO = 10                      # out-proj contraction tiles (1280 vp shard)
VP_SH = KO * 128             # 1280
RG = [[0, 1, 2, 3], [4, 5, 6, 7]]

AF = mybir.ActivationFunctionType
OP = mybir.AluOpType


def build_nc(sim_safe=False, debug_partial=False, skip_collective=False):
    nc = bacc.Bacc("TRN2", target_bir_lowering=False, debug=False,
                   num_devices=N_CORES)
    ap = {}
    ins_spec = [
        ("xt", [HID, L], BF16),
        ("wq", [KQ * 128, MTOT * 128], BF16),
        ("wo", [VP_SH, MO * 128], BF16),
        ("cos2", [128, L], BF16),
        ("sin2", [128, L], BF16),
        ("tri", [128, 128], BF16),
        ("lnw0", [128, 1], F32),
        ("lnb0", [128, 1], F32),
        ("b4", [128, MO], F32),
    ]
    for name, shape, dt in ins_spec:
        ap[name] = nc.dram_tensor(name, shape, dt, kind="ExternalInput").ap()
    out_sh = nc.dram_tensor("out_sh", [HID // TP, L], F32, kind="ExternalOutput").ap()
    if debug_partial:
        partial_dbg = nc.dram_tensor("partial", [HID, L], F32, kind="ExternalOutput").ap()

    with tile.TileContext(nc) as tc, ExitStack() as ctx:
        ctx.enter_context(nc.allow_low_precision(
            reason="bf16 compute pipeline by design; fp32 accumulation in PSUM"))
        wp = ctx.enter_context(tc.tile_pool(name="wp", bufs=1))
        xp = ctx.enter_context(tc.tile_pool(name="xp", bufs=2))
        bp = ctx.enter_context(tc.tile_pool(name="bp", bufs=3))
        tp_ = ctx.enter_context(tc.tile_pool(name="tp", bufs=2))
        rp = ctx.enter_context(tc.tile_pool(name="rp", bufs=1))
        pep = ctx.enter_context(tc.tile_pool(name="pep", bufs=4))
        stp = ctx.enter_context(tc.tile_pool(name="stp", bufs=3))
        psmm = ctx.enter_context(tc.tile_pool(name="psmm", bufs=2, space="PSUM"))
        psst = ctx.enter_context(tc.tile_pool(name="psst", bufs=1, space="PSUM"))
        pspt = ctx.enter_context(tc.tile_pool(name="pspt", bufs=2, space="PSUM"))
        psot = ctx.enter_context(tc.tile_pool(name="psot", bufs=2, space="PSUM"))
        dram = ctx.enter_context(tc.tile_pool(name="dram", bufs=1, space="DRAM"))

        # ---- prefetch x chunk 0 before the weight bulk so stats matmuls
        # and the first qkvp m-tile start as early as possible ----
        xt3 = ap["xt"].rearrange("(a p) t -> p a t", p=128)   # [128, 8, L]
        xc0 = xp.tile([128, 8 * CH], BF16, tag="xc", name="xc0")
        nc.gpsimd.dma_start(out=xc0[:].rearrange("p (a t) -> p a t", a=8),
                          in_=xt3[:, :, 0:CH])

        # ---- resident weights / constants ----
        # qk+v columns first: the first chunk's qk/v m-tiles can start while
        # the p columns and wo are still in flight.
        QKV_COLS = (MQK + MV) * 128  # 768
        wq3 = ap["wq"].rearrange("(a p) m -> p a m", p=128)    # [128, 9, 1792]
        wq_sb = wp.tile([128, KQ * MTOT * 128], BF16)   # [128, 9*1792]
        wq_sb3 = wq_sb[:].rearrange("p (a m) -> p a m", a=KQ)
        nc.gpsimd.dma_start(out=wq_sb3[:, :, 0:QKV_COLS], in_=wq3[:, :, 0:QKV_COLS])
        nc.gpsimd.dma_start(out=wq_sb3[:, :, QKV_COLS:], in_=wq3[:, :, QKV_COLS:])
        cos_sb = wp.tile([128, L], BF16)
        sin_sb = wp.tile([128, L], BF16)
        tri_sb = wp.tile([128, 128], BF16)
        nc.gpsimd.dma_start(out=cos_sb, in_=ap["cos2"])
        nc.gpsimd.dma_start(out=sin_sb, in_=ap["sin2"])
        nc.sync.dma_start(out=tri_sb, in_=ap["tri"])
        wo_sb = wp.tile([128, KO * MO * 128], BF16)     # [128, 10*1024]

        def load_wo():
            nc.gpsimd.dma_start(
                out=wo_sb[:].rearrange("p (a m) -> p a m", a=KO),
                in_=ap["wo"].rearrange("(a p) m -> p a m", p=128))
        lnw0 = wp.tile([128, 1], F32)
        lnb0 = wp.tile([128, 1], F32)
        b4_sb = wp.tile([128, MO], F32)
        nc.sync.dma_start(out=lnw0, in_=ap["lnw0"])
        nc.sync.dma_start(out=lnb0, in_=ap["lnb0"])
        nc.sync.dma_start(out=b4_sb, in_=ap["b4"])
        ones_sb = wp.tile([128, 1], BF16)
        nc.vector.memset(ones_sb, 1.0 / HID)
        ident = wp.tile([128, 128], BF16)
        make_identity(nc, ident)
        eps_sb = wp.tile([1, 1], F32)
        nc.vector.memset(eps_sb, 1e-5)
        carry = wp.tile([128, 1], F32)

        qk_t = [wp.tile([128, L], BF16, tag=f"qk{i}", name=f"qk{i}") for i in range(4)]  # qq01,kk01,qq23,kk23
        vaug = [wp.tile([128, NKB, 65], BF16, tag=f"v{h}", name=f"v{h}") for h in range(HPC)]
        for h in range(HPC):
            nc.vector.memset(vaug[h][:, :, 64:65], 1.0)
        pp = ctx.enter_context(tc.tile_pool(name="pp", bufs=2))

        bounce_in = [dram.tile([HID, CH], F32, name=f"rsin{c}") for c in range(NCH)]
        bounce_out = [dram.tile([HID // TP, CH], F32, name=f"rsout{c}") for c in range(NCH)]

        def pre_phase(c, xc):
            """Stats + soft-prefix scan + aug/partT rhs tiles for chunk c.

            Emitted one chunk AHEAD of its qkvp matmuls (between chunk c-1's
            attention and out-proj) so the stats->DVE->broadcast->aug chain
            is off the PE critical path at chunk boundaries.
            """
            # ---- stats: mean and mean-square via ones-matmul ----
            mu_ps = psst.tile([1, CH], F32, tag="st0", name=f"mu_ps{c}")
            sq_ps = psst.tile([1, CH], F32, tag="st1", name=f"sq_ps{c}")
            for kt in range(8):
                nc.tensor.matmul(mu_ps, ones_sb, xc[:, kt * CH:(kt + 1) * CH],
                                 start=(kt == 0), stop=(kt == 7))
            for kt in range(8):
                sq = xp.tile([128, CH], BF16, tag="sq", name=f"sq{c}_{kt}")
                nc.scalar.activation(out=sq, in_=xc[:, kt * CH:(kt + 1) * CH],
                                     func=AF.Square)
                nc.tensor.matmul(sq_ps, ones_sb, sq,
                                 start=(kt == 0), stop=(kt == 7))
            mu_row = rp.tile([1, CH], F32, tag="mu_row", bufs=2)
            nc.vector.tensor_copy(out=mu_row, in_=mu_ps)
            mu_bf = rp.tile([1, CH], BF16, tag="mu_bf", bufs=2)
            nc.vector.tensor_copy(out=mu_bf, in_=mu_ps)
            var_row = rp.tile([1, CH], F32, tag="var", bufs=2)
            nc.vector.scalar_tensor_tensor(out=var_row, in0=mu_row, scalar=-1.0,
                                           in1=mu_ps, op0=OP.mult, op1=OP.mult)
            nc.vector.tensor_add(out=var_row, in0=var_row, in1=sq_ps)
            sqv_row = rp.tile([1, CH], F32, tag="sqv", bufs=2)
            nc.scalar.activation(out=sqv_row, in_=var_row, func=AF.Sqrt,
                                 bias=eps_sb, scale=1.0)
            rstd_row = rp.tile([1, CH], F32, tag="rstd", bufs=2)
            nc.vector.reciprocal(out=rstd_row, in_=sqv_row)
            mu_b = bp.tile([128, CH], BF16, tag="mu_b", name=f"mu_b{c}")
            rstd_b = bp.tile([128, CH], F32, tag="rstd_b", name=f"rstd_b{c}")
            sqv_b = bp.tile([128, CH], F32, tag="sqv_b", name=f"sqv_b{c}")
            nc.gpsimd.partition_broadcast(mu_b, mu_bf)
            nc.gpsimd.partition_broadcast(rstd_b, rstd_row)
            nc.gpsimd.partition_broadcast(sqv_b, sqv_row)

            # ---- center x in place (channels 128..1023): x <- x - mu.
            # Replaces the aug contraction tile; runs off the PE critical
            # path thanks to the one-chunk-ahead pre-phase pipelining.
            for kt in range(1, 8):
                nc.vector.tensor_tensor(
                    out=xc[:, kt * CH:(kt + 1) * CH],
                    in0=xc[:, kt * CH:(kt + 1) * CH], in1=mu_b,
                    op=OP.subtract)

            # ---- soft prefix max on channels 0-127 ----
            h0 = tp_.tile([128, CH], F32, tag="h0", name=f"h0_{c}")
            nc.vector.tensor_tensor(out=h0, in0=xc[:, 0:CH], in1=mu_b,
                                    op=OP.subtract)
            nc.vector.tensor_mul(out=h0, in0=h0, in1=rstd_b)
            nc.vector.tensor_scalar(out=h0, in0=h0, scalar1=lnw0, scalar2=lnb0,
                                    op0=OP.mult, op1=OP.add)
            e0 = tp_.tile([128, CH], BF16, tag="e0", name=f"e0_{c}")
            nc.scalar.activation(out=e0, in_=h0, func=AF.Exp, scale=5.0)
            c0 = tp_.tile([128, CH], F32, tag="c0", name=f"c0_{c}")
            nc.vector.tensor_tensor_scan(
                out=c0, data0=e0, data1=e0,
                initial=(0.0 if c == 0 else carry[:, 0:1]),
                op0=OP.add, op1=OP.bypass)
            nc.vector.tensor_copy(out=carry, in_=c0[:, CH - 1:CH])
            lnc = tp_.tile([128, CH], BF16, tag="lnc", name=f"lnc{c}")
            nc.scalar.activation(out=lnc, in_=c0, func=AF.Ln)
            partT = tp_.tile([128, CH], BF16, tag="partT", name=f"partT{c}")
            nc.vector.tensor_mul(out=partT, in0=lnc, in1=sqv_b)
            return dict(partT=partT, rstd_b=rstd_b)

        xcs = {0: xc0}
        pres = {0: pre_phase(0, xc0)}
        for c in range(NCH):
            t0, t1 = c * CH, (c + 1) * CH
            xc = xcs[c]
            partT, rstd_b = pres[c]["partT"], pres[c]["rstd_b"]
            p_t = [pp.tile([128, CH], BF16, tag=f"p{i}", name=f"p{i}_{c}")
                   for i in range(MP)]
            cat01 = pp.tile([128, CH], BF16, tag="cat01", name=f"cat01_{c}")
            cat23 = pp.tile([128, CH], BF16, tag="cat23", name=f"cat23_{c}")

            # ---- qkvp projection: 14 m-tiles x 9 k-tiles ----
            rhs_tiles = [xc[:, kt * CH:(kt + 1) * CH] for kt in range(1, 8)]
            rhs_tiles += [partT]
            for mt in range(MTOT):
                mm = psmm.tile([128, CH], F32, tag="mm")
                for kt in range(KQ):
                    nc.tensor.matmul(
                        mm,
                        wq_sb[:, kt * 1792 + mt * 128: kt * 1792 + (mt + 1) * 128],
                        rhs_tiles[kt],
                        start=(kt == 0), stop=(kt == KQ - 1))
                if mt < MQK:
                    qks = tp_.tile([128, CH], BF16, tag="qks")
                    nc.vector.tensor_mul(out=qks, in0=mm, in1=rstd_b)
                    rot = tp_.tile([128, CH], BF16, tag="rot")
                    nc.vector.tensor_copy(out=rot[0:32], in_=qks[32:64])
                    nc.vector.tensor_copy(out=rot[32:64], in_=qks[0:32])
                    nc.vector.tensor_copy(out=rot[64:96], in_=qks[96:128])
                    nc.vector.tensor_copy(out=rot[96:128], in_=qks[64:96])
                    qc = tp_.tile([128, CH], BF16, tag="qc")
                    nc.vector.tensor_mul(out=qc, in0=qks, in1=cos_sb[:, t0:t1])
                    nc.vector.tensor_mul(out=rot, in0=rot, in1=sin_sb[:, t0:t1])
                    nc.vector.tensor_add(out=qk_t[mt][:, t0:t1], in0=qc, in1=rot)
                elif mt < MQK + MV:
                    vi = mt - MQK
                    v_sb = tp_.tile([128, CH], BF16, tag="v_sb")
                    nc.vector.tensor_mul(out=v_sb, in0=mm, in1=rstd_b)
                    for half in range(2):
                        h = 2 * vi + half
                        for blk in range(CH // KB):
                            jb = (CH // KB) * c + blk
                            tr = pspt.tile([128, 64], BF16, tag="pt")
                            nc.tensor.transpose(
                                tr, v_sb[64 * half:64 * half + 64,
                                         blk * KB:(blk + 1) * KB],
                                ident[64 * half:64 * half + 64,
                                      64 * half:64 * half + 64])
                            nc.vector.tensor_copy(out=vaug[h][:, jb, 0:64], in_=tr)
                else:
                    pi = mt - MQK - MV
                    pf = tp_.tile([128, CH], BF16, tag="pf")
                    nc.vector.tensor_mul(out=pf, in0=mm, in1=rstd_b)
                    if sim_safe:
                        sg = tp_.tile([128, CH], BF16, tag="sg")
                        nc.scalar.activation(out=sg, in_=pf, func=AF.Sigmoid,
                                             scale=1.702)
                        nc.vector.tensor_mul(out=p_t[pi], in0=pf, in1=sg)
                    else:
                        nc.scalar.activation(out=p_t[pi], in_=pf, func=AF.Gelu)

            if c == 0:
                load_wo()

            # ---- attention for this q-chunk, two heads at a time ----
            # Heads of a pair use PE row groups 0-63 / 64-127, so their PT
            # matmuls run concurrently in the array.
            nblk = (CH // KB) * (c + 1)
            for pair in range(HPC // 2):
                qq = qk_t[2 * pair]
                kk = qk_t[2 * pair + 1]
                ots = [psot.tile([65, CH], F32, tag="ot", name=f"ot{c}_{pair}_{i}")
                       for i in range(2)]
                for j in range(nblk):
                    dm = j - (CH // KB) * c
                    qlo = KB * dm if dm >= 0 else 0
                    pts, pes = [], []
                    for i in range(2):
                        sl = slice(64 * i, 64 * i + 64)
                        pt = pspt.tile([128, CH], F32, tag="pt",
                                       name=f"pt{c}_{pair}_{j}_{i}")
                        nc.tensor.matmul(
                            pt[:, qlo:CH],
                            kk[sl, j * KB:(j + 1) * KB],
                            qq[sl, t0 + qlo:t1],
                            start=True, stop=True)
                        pts.append(pt)
                    for i in range(2):
                        pe = pep.tile([128, CH], BF16, tag="pe",
                                      name=f"pe{c}_{pair}_{j}_{i}")
                        nc.scalar.activation(out=pe[:, qlo:CH],
                                             in_=pts[i][:, qlo:CH], func=AF.Exp)
                        if dm >= 0:
                            nc.vector.tensor_mul(out=pe[:, qlo:qlo + KB],
                                                 in0=pe[:, qlo:qlo + KB],
                                                 in1=tri_sb)
                        pes.append(pe)
                    for i in range(2):
                        h = 2 * pair + i
                        nc.tensor.matmul(
                            ots[i][:, qlo:CH], vaug[h][:, j, :], pes[i][:, qlo:CH],
                            start=(j == 0), stop=(j == nblk - 1),
                            skip_group_check=True)
                for i in range(2):
                    h = 2 * pair + i
                    ot = ots[i]
                    den = rp.tile([1, CH], BF16, tag="den", bufs=2)
                    nc.vector.reciprocal(out=den, in_=ot[64:65, :])
                    den_b = bp.tile([64, CH], BF16, tag="den_b")
                    nc.gpsimd.partition_broadcast(den_b, den)
                    dest = cat01 if h < 2 else cat23
                    r0 = 64 * (h % 2)
                    nc.vector.tensor_mul(out=dest[r0:r0 + 64, :],
                                         in0=ot[0:64, :], in1=den_b)

            # ---- pipeline chunk c+1's x load and pre-phase here, so its
            # stats/scan/broadcast chain overlaps this chunk's out-proj ----
            if c + 1 < NCH:
                xn = xp.tile([128, 8 * CH], BF16, tag="xc", name=f"xc{c + 1}")
                nc.gpsimd.dma_start(out=xn[:].rearrange("p (a t) -> p a t", a=8),
                                    in_=xt3[:, :, (c + 1) * CH:(c + 2) * CH])
                xcs[c + 1] = xn
                pres[c + 1] = pre_phase(c + 1, xn)

            # ---- out-proj for this chunk: 8 m-tiles x 10 k-tiles ----
            orhs = [cat01, cat23] + [p_t[i] for i in range(MP)]
            kt_order = list(range(2, KO)) + [0, 1]   # p first, cat last
            bn3 = bounce_in[c][:].rearrange("(a p) t -> p a t", p=128)
            for mg in range(MO // 2):
                st = stp.tile([128, 2 * CH], F32, tag="st")
                for mi in range(2):
                    mt = 2 * mg + mi
                    mm = psmm.tile([128, CH], F32, tag="mm")
                    for ki, kt in enumerate(kt_order):
                        nc.tensor.matmul(
                            mm,
                            wo_sb[:, kt * 1024 + mt * 128: kt * 1024 + (mt + 1) * 128],
                            orhs[kt],
                            start=(ki == 0), stop=(ki == KO - 1))
                    nc.scalar.activation(out=st[:, mi * CH:(mi + 1) * CH],
                                         in_=mm, func=AF.Identity,
                                         bias=b4_sb[:, mt:mt + 1], scale=1.0)
                nc.gpsimd.dma_start(
                    out=bn3[:, 2 * mg:2 * mg + 2, :],
                    in_=st[:].rearrange("p (a t) -> p a t", a=2))

            # ---- chunked reduce-scatter: overlap comm with later chunks ----
            if not skip_collective:
                nc.gpsimd.collective_compute(
                    "ReduceScatter", OP.add,
                    ins=[bounce_in[c][:].opt()],
                    outs=[bounce_out[c][:].opt()],
                    replica_groups=RG)
                nc.gpsimd.dma_start(out=out_sh[:, t0:t1],
                                    in_=bounce_out[c][:])

        if debug_partial:
            for c in range(NCH):
                nc.sync.dma_start(out=partial_dbg[:, c * CH:(c + 1) * CH],
                                  in_=bounce_in[c][:])
    nc.compile()
    return nc


# ---------------- host-side sharding ----------------

def _rope_tables():
    inv = 1.0 / (10000.0 ** (np.arange(0, D, 2, dtype=np.float64) / D))
    t = np.arange(L, dtype=np.float64)
    f = t[:, None] * inv[None, :]                 # [L, 32]
    emb = np.concatenate([f, f], axis=1)          # [L, 64]
    cos64 = np.cos(emb).T                         # [64, L]
    sin64 = np.sin(emb).T
    s32 = sin64[0:32]
    sin_signed = np.concatenate([-s32, s32], axis=0)   # [64, L]
    cos2 = np.concatenate([cos64, cos64], axis=0)
    sin2 = np.concatenate([sin_signed, sin_signed], axis=0)
    bf = ml_dtypes.bfloat16
    return cos2.astype(bf), sin2.astype(bf)


def prep_inputs(x, ln_w, ln_b, w_in, w_out, b_out):
    x = np.asarray(x, np.float32)
    ln_w = np.asarray(ln_w, np.float32)
    ln_b = np.asarray(ln_b, np.float32)
    w_in = np.asarray(w_in, np.float32)
    w_out = np.asarray(w_out, np.float32)
    b_out = np.asarray(b_out, np.float32)

    cos2, sin2 = _rope_tables()
    tri = (np.arange(128)[None, :] >= np.arange(128)[:, None]).astype(ml_dtypes.bfloat16)
    lnw0 = ln_w[0:128, None].astype(np.float32)
    lnb0 = ln_b[0:128, None].astype(np.float32)
    b4 = (b_out / TP).reshape(MO, 128).T.astype(np.float32).copy()

    xt_b = [np.ascontiguousarray(x[b].T).astype(ml_dtypes.bfloat16) for b in range(B)]

    in_maps = []
    for c in range(N_CORES):
        b, tpi = divmod(c, TP)
        heads = [HPC * tpi + j for j in range(HPC)]
        # qkvp shard row order: q0,q1, k0,k1, q2,q3, k2,k3, v0..v3, p(1024)
        rows = []
        for pair in range(HPC // 2):
            h0, h1 = heads[2 * pair], heads[2 * pair + 1]
            rows += list(range(64 * h0, 64 * h0 + 64))
            rows += list(range(64 * h1, 64 * h1 + 64))
            rows += list(range(HID + 64 * h0, HID + 64 * h0 + 64))
            rows += list(range(HID + 64 * h1, HID + 64 * h1 + 64))
        for h in heads:
            rows += list(range(2 * HID + 64 * h, 2 * HID + 64 * h + 64))  # v_h
        rows += list(range(3 * HID + 1024 * tpi, 3 * HID + 1024 * (tpi + 1)))
        rows = np.array(rows)
        W_sh = w_in[rows, :]                                    # [1792, 1024]
        w_eff = W_sh * ln_w[None, :]
        c1 = W_sh[:, ACC:] @ ln_b[ACC:]
        assert np.abs(c1).max() < 1e-6, (
            "nonzero ln_b[128:] not supported by this build (c1 term dropped)")
        wq = np.zeros((KQ * 128, MTOT * 128), np.float32)
        wq[0:896] = w_eff[:, ACC:].T                            # channels 128..1023
        wq[896:1024] = 0.2 * W_sh[:, 0:ACC].T
        qs = 1.0 / float(D) ** 0.5                # fold q * D^-1/2 into weights
        wq[:, 0:128] *= qs
        wq[:, 256:384] *= qs
        # out-proj shard: columns [256*tpi:256*(tpi+1)] (o) + [1024+1024*tpi ...] (p)
        ocols = list(range(256 * tpi, 256 * (tpi + 1)))
        pcols = list(range(HID + 1024 * tpi, HID + 1024 * (tpi + 1)))
        wo = w_out[:, ocols + pcols].T                          # [1280, 1024]
        in_maps.append({
            "xt": xt_b[b],
            "wq": wq.astype(ml_dtypes.bfloat16),
            "wo": np.ascontiguousarray(wo).astype(ml_dtypes.bfloat16),
            "cos2": cos2, "sin2": sin2, "tri": tri,
            "lnw0": lnw0, "lnb0": lnb0, "b4": b4,
        })
    return in_maps


def assemble(results):
    """results: list of 8 per-core dicts with 'out_sh' [256, L] f32."""
    out = np.empty((B, L, HID), np.float32)
    for c in range(N_CORES):
        b, tpi = divmod(c, TP)
        out[b, :, 256 * tpi:256 * (tpi + 1)] = results[c]["out_sh"].T
    return out


_NC_CACHE = {}


def _get_nc():
    if "nc" not in _NC_CACHE:
        _NC_CACHE["nc"] = build_nc()
    return _NC_CACHE["nc"]


def kernel(x, ln_w, ln_b, w_in, w_out, b_out):
    from concourse.bass_utils import run_bass_kernel_spmd
    in_maps = prep_inputs(x, ln_w, ln_b, w_in, w_out, b_out)
    nc = _get_nc()
    res = run_bass_kernel_spmd(nc, in_maps, core_ids=list(range(N_CORES)))
    return assemble(res.results)



# revision 66
# speedup vs baseline: 1.1614x; 1.1614x over previous
"""Trainium2 Bass kernel for nn_Block_56427280335230 (dense transformer block).

Reference semantics (B=2, L=2048, H=16, D=64, HID=1024):
    h = LayerNorm(x) * ln_w + ln_b
    h[..., :128] = cumlogsumexp(h[..., :128] * 5, axis=seq) / 5
    qkvp = h @ w_in.T ; split q,k,v,p
    q,k = rope(q), rope(k)
    o = softmax(q k^T / 8 + causal) v
    out = concat([o, gelu(p)]) @ w_out.T + b_out
    (b_out = 0, ln_b = 0, ln_w = 1 asserted host-side where folded)

Sharding: DP2 x TP4 over 8 NeuronCores. Cores 0-3 take batch 0, cores 4-7
batch 1. Within a group of 4, heads (4 per core) and the qkvp/vp weight
columns are sharded. Each core computes a full partial out^T [1024, 2048]
over its vp shard; a ReduceScatter over the 4-core group leaves each core
with a disjoint 256-channel slice of the summed output.

On-device dataflow is feature-major (channels on partitions, tokens on the
free axis). LayerNorm: token stats via ones-matmuls; the centered+rstd-scaled
activation tiles feed every downstream matmul (so no per-output rescale).
The soft-prefix-max uses the DVE prefix-scan; its ln() output feeds the
matmuls directly as an extra contraction tile. rstd = exp(-0.5 ln(var+eps))
keeps the Activation engine inside one act-table set (exp/ln/square/identity)
for most of the chunk; only Gelu forces one extra set swap per chunk.
V is produced pre-transposed ([keys, d]) by contracting h-tiles as lhsT
against w_v columns, so no PE transposes are needed. Attention computes S^T
blocks (keys on partitions); exp(S^T) is written in fp8e4m3; the AV matmul
runs in DoubleRow fp8 over key-block pairs with an fp8 residual term
(v - fp8(v)) restoring v to bf16 accuracy; the softmax denominator rides as
a ones-column. All other matmuls are bf16 with fp32 accumulation.
"""
import numpy as np
import ml_dtypes
from contextlib import ExitStack, nullcontext

from concourse import bass, mybir, tile, bacc

F32 = mybir.dt.float32
BF16 = mybir.dt.bfloat16
FP8 = mybir.dt.float8e4

B, L, H, D = 2, 2048, 16, 64
HID = H * D                  # 1024
ACC = HID // 8               # 128 scan channels
N_CORES = 8
TP = 4                       # tensor-parallel group size
HPC = H // TP                # heads per core = 4
CH = 512                     # tokens per chunk
NCH = L // CH                # 4 chunks
KB = 128                     # key block
NKB = L // KB                # 16 key blocks
KQ = 8                       # qkvp contraction tiles: 7 centered-x + 1 part
MQK, MP = HPC, 8
MTOT = MQK + MP              # 12 m-tiles of the qkvp output (q/k + p rows)
MO = 8                       # out-proj m-tiles (1024 out channels)
KO = 10                      # out-proj contraction tiles (1280 vp shard)
VP_SH = KO * 128             # 1280
RG = [[0, 1, 2, 3], [4, 5, 6, 7]]

AF = mybir.ActivationFunctionType
OP = mybir.AluOpType
PM = mybir.MatmulPerfMode


def build_nc(sim_safe=False, debug_partial=False, skip_collective=False,
             boosts=frozenset({"den", "h8", "qks", "scan", "st", "vaug"})):
    nc = bacc.Bacc("TRN2", target_bir_lowering=False, debug=False,
                   num_devices=N_CORES)
    ap = {}
    ins_spec = [
        ("xt", [HID, L], BF16),
        ("wq", [KQ * 128, MP * 128], BF16),
        ("wq8", [KQ * 128, MQK * 128], FP8),
        ("wv", [KQ * 128, HPC * 64], BF16),
        ("wo", [(KO - 2) * 128, MO * 128], BF16),
        ("wo8", [2 * 128, MO * 128], BF16),
        ("cos2", [128, L], BF16),
        ("sin2", [128, L], BF16),
        ("tri", [128, 128], BF16),
        ("lnw0", [128, 1], F32),
        ("lnb0", [128, 1], F32),
    ]
    for name, shape, dt in ins_spec:
        ap[name] = nc.dram_tensor(name, shape, dt, kind="ExternalInput").ap()
    out_sh = nc.dram_tensor("out_sh", [HID // TP, L], F32, kind="ExternalOutput").ap()
    if debug_partial:
        partial_dbg = nc.dram_tensor("partial", [HID, L], F32, kind="ExternalOutput").ap()

    sim_safe_flag = [sim_safe]
    with tile.TileContext(nc) as tc, ExitStack() as ctx:
        ctx.enter_context(nc.allow_low_precision(
            reason="bf16/fp8 compute pipeline by design; fp32 accumulation in PSUM"))
        wp = ctx.enter_context(tc.tile_pool(name="wp", bufs=1))
        xp = ctx.enter_context(tc.tile_pool(name="xp", bufs=2))
        bp = ctx.enter_context(tc.tile_pool(name="bp", bufs=3))
        tp_ = ctx.enter_context(tc.tile_pool(name="tp", bufs=2))
        rp = ctx.enter_context(tc.tile_pool(name="rp", bufs=1))
        pep = ctx.enter_context(tc.tile_pool(name="pep", bufs=4))
        stp = ctx.enter_context(tc.tile_pool(name="stp", bufs=3))
        psmm = ctx.enter_context(tc.tile_pool(name="psmm", bufs=3, space="PSUM"))
        psst = ctx.enter_context(tc.tile_pool(name="psst", bufs=1, space="PSUM"))
        pspt = ctx.enter_context(tc.tile_pool(name="pspt", bufs=2, space="PSUM"))
        psot = ctx.enter_context(tc.tile_pool(name="psot", bufs=2, space="PSUM"))
        dram = ctx.enter_context(tc.tile_pool(name="dram", bufs=1, space="DRAM"))

        # ---- prefetch x chunk 0 before the weight bulk ----
        xt3 = ap["xt"].rearrange("(a p) t -> p a t", p=128)   # [128, 8, L]
        xc0 = xp.tile([128, 8 * CH], BF16, tag="xc", name="xc0")
        xc03 = xc0[:].rearrange("p (a t) -> p a t", a=8)
        nc.sync.dma_start(out=xc03[:, 0:1], in_=xt3[:, 0:1, 0:CH])
        nc.sync.dma_start(out=xc03[:, 1:2], in_=xt3[:, 1:2, 0:CH])
        nc.sync.dma_start(out=xc03[:, 2:8], in_=xt3[:, 2:8, 0:CH])

        # ---- resident weights / constants ----
        wq_sb = wp.tile([128, KQ, MP * 128], BF16)      # p rows only
        nc.gpsimd.dma_start(out=wq_sb,
                            in_=ap["wq"].rearrange("(a p) m -> p a m", p=128))
        # qk k-pairs, x32 scaled, m-tile-major so each [128, 2, 128] weight
        # slice is contiguous (Ldweights requires packed DoubleRow halves)
        wq8_sb = wp.tile([128, MQK, 4, 2, 128], FP8)
        nc.gpsimd.dma_start(
            out=wq8_sb,
            in_=ap["wq8"].rearrange("(pr two p) (m f) -> p m pr two f",
                                    pr=4, p=128, m=MQK))
        wv_sb = wp.tile([128, KQ, HPC * 64], BF16)      # [128, 8, 256]
        nc.gpsimd.dma_start(out=wv_sb,
                            in_=ap["wv"].rearrange("(a p) m -> p a m", p=128))
        cos_sb = wp.tile([128, L], BF16)
        sin_sb = wp.tile([128, L], BF16)
        tri_sb = wp.tile([128, 128], BF16)
        nc.gpsimd.dma_start(out=cos_sb, in_=ap["cos2"])
        nc.gpsimd.dma_start(out=sin_sb, in_=ap["sin2"])
        nc.sync.dma_start(out=tri_sb, in_=ap["tri"])
        wo_sb = wp.tile([128, KO - 2, MO * 128], BF16)  # p rows, x32 scaled
        wo8_sb = wp.tile([128, 2, MO * 128], BF16)      # cat rows, x32 scaled

        def load_wo():
            nc.gpsimd.dma_start(
                out=wo_sb,
                in_=ap["wo"].rearrange("(a p) m -> p a m", p=128))
            nc.gpsimd.dma_start(
                out=wo8_sb,
                in_=ap["wo8"].rearrange("(two p) m -> p two m", p=128))
        lnw0 = wp.tile([128, 1], F32)
        lnb0 = wp.tile([128, 1], F32)
        nc.sync.dma_start(out=lnw0, in_=ap["lnw0"])
        nc.sync.dma_start(out=lnb0, in_=ap["lnb0"])
        ones_sb = wp.tile([128, 1], BF16)
        nc.vector.memset(ones_sb, 1.0 / HID)
        eps_sb = wp.tile([1, 1], F32)
        nc.vector.memset(eps_sb, 1e-5)
        carry = wp.tile([128, 1], F32)
        # [zeros(128) | tri(128)] mask for the second half of a diagonal pair
        ztri_sb = wp.tile([128, 2 * KB], BF16)
        nc.vector.memset(ztri_sb[:, 0:KB], 0.0)
        nc.vector.tensor_copy(out=ztri_sb[:, KB:], in_=tri_sb)

        qk_t = [wp.tile([128, L], BF16, tag=f"qk{i}", name=f"qk{i}") for i in range(4)]  # qq01,kk01,qq23,kk23
        # vaug[h]: [keys, jpair, half, d+1] fp8; eps residual restores bf16 v
        # per-half column count must be a legal PE tile size (64/128), so
        # the 64 v-columns + ones-column are padded out to 128
        vaug = [wp.tile([128, NKB // 2, 2, 128], FP8, tag=f"v{h}", name=f"v{h}")
                for h in range(HPC)]
        vres = [wp.tile([128, NKB // 2, 2, 128], FP8, tag=f"vr{h}", name=f"vr{h}")
                for h in range(HPC)]
        for h in range(HPC):
            nc.vector.memset(vaug[h][:, :, :, 64:65], 1.0)
            nc.vector.memset(vaug[h][:, :, :, 65:128], 0.0)
            nc.vector.memset(vres[h][:, :, :, 64:128], 0.0)
        pp = ctx.enter_context(tc.tile_pool(name="pp", bufs=2))

        bounce_in = [dram.tile([HID, CH], F32, name=f"rsin{c}") for c in range(NCH)]
        bounce_out = [dram.tile([HID // TP, CH], F32, name=f"rsout{c}") for c in range(NCH)]

        def pre_phase_a(c, xc):
            """Stats + rstd + normalize-in-place + scan-exp for chunk c.

            Emitted one chunk AHEAD of its qkvp matmuls so the
            stats->DVE->broadcast chain is off the PE critical path. The Act
            sequence is Ln (lnv) then Exp (rstd, e0) so the following
            attention exps stay in the already-loaded exp table set.
            """
            # ---- stats: mean and mean-square via ones-matmuls ----
            # mu and sq share one PSUM slot (tag "st"); the sq chain starts
            # only after the mu row has been read out.
            mu_ps = psst.tile([1, CH], F32, tag="st", name=f"mu_ps{c}")
            for kt in range(8):
                nc.tensor.matmul(mu_ps, ones_sb, xc[:, kt * CH:(kt + 1) * CH],
                                 start=(kt == 0), stop=(kt == 7))
            mu_row = rp.tile([1, CH], F32, tag="mu_row", bufs=2)
            nc.vector.tensor_copy(out=mu_row, in_=mu_ps)
            mu_bf = rp.tile([1, CH], BF16, tag="mu_bf", bufs=2)
            nc.vector.tensor_copy(out=mu_bf, in_=mu_ps)
            var_row = rp.tile([1, CH], F32, tag="var", bufs=2)
            nc.vector.scalar_tensor_tensor(out=var_row, in0=mu_row, scalar=-1.0,
                                           in1=mu_ps, op0=OP.mult, op1=OP.mult)
            sq_ps = psst.tile([1, CH], F32, tag="st", name=f"sq_ps{c}")
            for kt in range(8):
                sq = xp.tile([128, CH], BF16, tag="sq", name=f"sq{c}_{kt}")
                sl = xc[:, kt * CH:(kt + 1) * CH]
                nc.vector.tensor_tensor(out=sq, in0=sl, in1=sl, op=OP.mult)
                nc.tensor.matmul(sq_ps, ones_sb, sq,
                                 start=(kt == 0), stop=(kt == 7))
            nc.vector.tensor_add(out=var_row, in0=var_row, in1=sq_ps)
            # rstd = exp(-0.5 * ln(var + eps)) -- stays in the exp/ln act sets
            lnv_row = rp.tile([1, CH], F32, tag="lnv", bufs=2)
            nc.scalar.activation(out=lnv_row, in_=var_row, func=AF.Ln,
                                 bias=eps_sb, scale=1.0)
            rstd_row = rp.tile([1, CH], BF16, tag="rstd", bufs=2)
            nc.scalar.activation(out=rstd_row, in_=lnv_row, func=AF.Exp,
                                 scale=-0.5)
            mu_b = bp.tile([128, CH], BF16, tag="mu_b", name=f"mu_b{c}")
            rstd_b = bp.tile([128, CH], BF16, tag="rstd_b", name=f"rstd_b{c}")
            nc.gpsimd.partition_broadcast(mu_b, mu_bf)
            nc.gpsimd.partition_broadcast(rstd_b, rstd_row)

            # ---- normalize x in place: x <- (x - mu) * rstd; quantize the
            # qk-contraction tiles (x k-tiles 1-7) to paired fp8 on Pool;
            # the 4th pair's second half (partT) is filled in pre_phase_b ----
            h8 = [bp.tile([128, 2, CH], FP8, tag=f"h8{pr}", bufs=2,
                          name=f"h8_{c}_{pr}") for pr in range(4)]
            for kt in range(8):
                sl = xc[:, kt * CH:(kt + 1) * CH]
                nc.vector.tensor_tensor(out=sl, in0=sl, in1=mu_b, op=OP.subtract)
                nc.vector.tensor_tensor(out=sl, in0=sl, in1=rstd_b, op=OP.mult)
                if kt >= 1:
                    pr, half = divmod(kt - 1, 2)
                    with (tc.high_priority() if "h8" in boosts else nullcontext()):
                        nc.gpsimd.tensor_copy(out=h8[pr][:, half, :], in_=sl)
            h8s[c] = h8

            # ---- soft prefix max, exp part ----
            h0 = tp_.tile([128, CH], F32, tag="h0", name=f"h0_{c}")
            nc.vector.tensor_scalar(out=h0, in0=xc[:, 0:CH], scalar1=lnw0,
                                    scalar2=lnb0, op0=OP.mult, op1=OP.add)
            e0 = tp_.tile([128, CH], BF16, tag="e0", name=f"e0_{c}")
            nc.scalar.activation(out=e0, in_=h0, func=AF.Exp, scale=5.0)
            return e0

        def pre_phase_b(c, e0):
            """Prefix-scan cumsum + ln for chunk c (emitted after chunk c-1's
            attention so its Ln lands after the exp block)."""
            c0 = tp_.tile([128, CH], F32, tag="c0", name=f"c0_{c}")
            with (tc.high_priority() if "scan" in boosts else nullcontext()):
                nc.vector.tensor_tensor_scan(
                    out=c0, data0=e0, data1=e0,
                    initial=(0.0 if c == 0 else carry[:, 0:1]),
                    op0=OP.add, op1=OP.bypass)
                nc.vector.tensor_copy(out=carry, in_=c0[:, CH - 1:CH])
            partT = tp_.tile([128, CH], BF16, tag="partT", name=f"partT{c}")
            with (tc.high_priority() if "scan" in boosts else nullcontext()):
                nc.scalar.activation(out=partT, in_=c0, func=AF.Ln)
            with (tc.high_priority() if "h8" in boosts else nullcontext()):
                nc.gpsimd.tensor_copy(out=h8s[c][3][:, 1, :], in_=partT)
            # prefetch the gelu act table right after the scan's Ln, while
            # Act is otherwise idle, so the next chunk's first real gelu
            # doesn't eat the table load in the PSUM-drain path
            dge = rp.tile([1, 1], BF16, tag="dge", bufs=2, name=f"dge{c}")
            nc.scalar.activation(out=dge, in_=eps_sb,
                                 func=(AF.Sigmoid if sim_safe_flag[0] else AF.Gelu))
            return partT

        xcs = {0: xc0}
        h8s = {}
        pres = {0: pre_phase_b(0, pre_phase_a(0, xc0))}
        pending = {}
        for c in range(NCH):
            t0, t1 = c * CH, (c + 1) * CH
            xc = xcs[c]
            partT = pres[c]

            # kick chunk c+1's x load immediately; its stats run after the
            # p-tiles below and its scan after this chunk's attention.
            if c + 1 < NCH:
                xn = xp.tile([128, 8 * CH], BF16, tag="xc", name=f"xc{c + 1}")
                nc.sync.dma_start(out=xn[:].rearrange("p (a t) -> p a t", a=8),
                                  in_=xt3[:, :, (c + 1) * CH:(c + 2) * CH])
                xcs[c + 1] = xn
            p_t = [pp.tile([128, CH], BF16, tag=f"p{i}", name=f"p{i}_{c}")
                   for i in range(MP)]
            cat8 = pp.tile([128, 2, CH], BF16, tag="cat8", name=f"cat8_{c}")

            rhs_tiles = [xc[:, kt * CH:(kt + 1) * CH] for kt in range(1, 8)]
            rhs_tiles += [partT]

            # ---- qk m-tiles: 4 fp8-DR k-pairs (x32-scaled weights), then
            # rope; the 1/32 descale rides the PSUM copy ----
            for mt in range(MQK):
                mm = psmm.tile([128, CH], F32, tag="mm")
                for pr in range(4):
                    nc.tensor.matmul(
                        mm, wq8_sb[:, mt, pr],
                        h8s[c][pr],
                        start=(pr == 0), stop=(pr == 3),
                        perf_mode=PM.DoubleRow, skip_group_check=True)
                qks = tp_.tile([128, CH], BF16, tag="qks")
                with (tc.high_priority() if "qks" in boosts else nullcontext()):
                    nc.vector.tensor_scalar(out=qks, in0=mm, scalar1=1.0 / 32,
                                            scalar2=None, op0=OP.mult)
                rot = tp_.tile([128, CH], BF16, tag="rot")
                nc.vector.tensor_copy(out=rot[0:32], in_=qks[32:64])
                nc.vector.tensor_copy(out=rot[32:64], in_=qks[0:32])
                nc.vector.tensor_copy(out=rot[64:96], in_=qks[96:128])
                nc.vector.tensor_copy(out=rot[96:128], in_=qks[64:96])
                qc = tp_.tile([128, CH], BF16, tag="qc")
                nc.vector.tensor_mul(out=qc, in0=qks, in1=cos_sb[:, t0:t1])
                nc.vector.tensor_mul(out=rot, in0=rot, in1=sin_sb[:, t0:t1])
                nc.vector.tensor_add(out=qk_t[mt][:, t0:t1], in0=qc, in1=rot)

            # ---- v pre-transposed: per key block, contract h-tiles as lhsT
            # against w_v columns -> [tokens, (head, d)] ----
            for jb in range(CH // KB):
                j = (CH // KB) * c + jb
                jp, jh = divmod(j, 2)
                vps = psmm.tile([128, HPC * 64], F32, tag="mm", name=f"vps{c}_{jb}")
                for kt in range(KQ):
                    nc.tensor.matmul(
                        vps,
                        rhs_tiles[kt][:, jb * KB:(jb + 1) * KB],
                        wv_sb[:, kt, :],
                        start=(kt == 0), stop=(kt == KQ - 1))
                with (tc.high_priority() if "vaug" in boosts else nullcontext()):
                    v_bf = tp_.tile([128, HPC * 64], BF16, tag="v_bf",
                                    name=f"vbf{c}_{jb}")
                    nc.vector.tensor_copy(out=v_bf, in_=vps)
                    for h in range(HPC):
                        nc.gpsimd.tensor_copy(out=vaug[h][:, jp, jh, 0:64],
                                              in_=v_bf[:, h * 64:(h + 1) * 64])
                        nc.gpsimd.tensor_tensor(
                            out=vres[h][:, jp, jh, 0:64],
                            in0=v_bf[:, h * 64:(h + 1) * 64],
                            in1=vaug[h][:, jp, jh, 0:64],
                            op=OP.subtract)

            # chunk c+1 stats/normalize/h8: emitted right after the qk chains
            # so the h8/partT feed chains finish well before chunk c+1 needs
            # them (stats matmuls slot into PE gaps during this chunk).
            if c + 1 < NCH and "early_pre_a" in boosts:
                e0n = pre_phase_a(c + 1, xcs[c + 1])



            # ---- p m-tiles: gelu straight from PSUM ----
            for pi in range(MP):
                mm = psmm.tile([128, CH], F32, tag="mm")
                for kt in range(KQ):
                    nc.tensor.matmul(
                        mm,
                        wq_sb[:, kt, pi * 128:(pi + 1) * 128],
                        rhs_tiles[kt],
                        start=(kt == 0), stop=(kt == KQ - 1))
                if sim_safe:
                    sg = tp_.tile([128, CH], BF16, tag="sg")
                    nc.scalar.activation(out=sg, in_=mm, func=AF.Sigmoid,
                                         scale=1.702)
                    nc.vector.tensor_tensor(out=p_t[pi], in0=mm, in1=sg,
                                            op=OP.mult)
                else:
                    nc.scalar.activation(out=p_t[pi], in_=mm, func=AF.Gelu)

            if c == 0:
                load_wo()

            # chunk c+1 stats/normalize/exp: its Ln+Exp land before this
            # chunk's attention exps in the Act queue (one table trip).
            if c + 1 < NCH and "early_pre_a" not in boosts:
                e0n = pre_phase_a(c + 1, xcs[c + 1])

            # ---- attention for this q-chunk, two heads at a time ----
            # pe written fp8; AV runs DoubleRow fp8 over key-block pairs
            # with an fp8 v-residual term restoring bf16-accurate v.
            npair = (CH // KB) * (c + 1) // 2
            for pair in range(HPC // 2):
                qq = qk_t[2 * pair]
                kk = qk_t[2 * pair + 1]
                ots = [psot.tile([128, CH], F32, tag="ot", name=f"ot{c}_{pair}_{i}")
                       for i in range(2)]
                for p2 in range(npair):
                    dm = 2 * p2 - (CH // KB) * c       # dm of first block in pair
                    qlo = KB * dm if dm >= 0 else 0
                    pes = []
                    for i in range(2):
                        sl = slice(64 * i, 64 * i + 64)
                        pe = pep.tile([128, 2, CH], FP8, tag="pe",
                                      name=f"pe{c}_{pair}_{p2}_{i}")
                        for jh in range(2):
                            j = 2 * p2 + jh
                            pt = pspt.tile([128, CH], F32, tag="pt",
                                           name=f"pt{c}_{pair}_{p2}_{i}_{jh}")
                            nc.tensor.matmul(
                                pt[:, qlo:CH],
                                kk[sl, j * KB:(j + 1) * KB],
                                qq[sl, t0 + qlo:t1],
                                start=True, stop=True)
                            nc.scalar.activation(out=pe[:, jh, qlo:CH],
                                                 in_=pt[:, qlo:CH], func=AF.Exp)
                        if dm >= 0:
                            # first half: tri on its diagonal strip; second
                            # half: zero the strip below its diagonal + tri.
                            nc.vector.tensor_mul(out=pe[:, 0, qlo:qlo + KB],
                                                 in0=pe[:, 0, qlo:qlo + KB],
                                                 in1=tri_sb)
                            nc.vector.tensor_mul(
                                out=pe[:, 1, qlo:qlo + 2 * KB],
                                in0=pe[:, 1, qlo:qlo + 2 * KB],
                                in1=ztri_sb)
                        pes.append(pe)
                    for i in range(2):
                        h = 2 * pair + i
                        nc.tensor.matmul(
                            ots[i][:, qlo:CH], vaug[h][:, p2], pes[i][:, :, qlo:CH],
                            start=(p2 == 0), stop=False,
                            perf_mode=PM.DoubleRow, skip_group_check=True)
                        nc.tensor.matmul(
                            ots[i][:, qlo:CH], vres[h][:, p2], pes[i][:, :, qlo:CH],
                            start=False, stop=(p2 == npair - 1),
                            perf_mode=PM.DoubleRow, skip_group_check=True)
                with (tc.high_priority() if "den" in boosts else nullcontext()):
                    for i in range(2):
                        h = 2 * pair + i
                        ot = ots[i]
                        den = rp.tile([1, CH], BF16, tag="den", bufs=2)
                        nc.vector.reciprocal(out=den, in_=ot[64:65, :])
                        den_b = bp.tile([64, CH], BF16, tag="den_b")
                        nc.gpsimd.partition_broadcast(den_b, den)
                        r0 = 64 * (h % 2)
                        nc.vector.tensor_mul(out=cat8[r0:r0 + 64, pair, :],
                                             in0=ot[0:64, :], in1=den_b)

                # previous chunk's deferred out-proj chains slot into this
                # Act-bound attention stretch to keep PE busy
                if c - 1 in pending:
                    pending[c - 1]([4 * pair + i for i in range(4)])
                    if pair == HPC // 2 - 1:
                        pending.pop(c - 1)

            # ---- chunk c+1 prefix-scan cumsum + ln ----
            if c + 1 < NCH:
                pres[c + 1] = pre_phase_b(c + 1, e0n)

            # out-proj + reduce-scatter for chunk c is DEFERRED to after
            # chunk c+1's qk/v matmuls: the den chain finishes meanwhile and
            # the OOO scheduler backfills attention with these chains. The
            # whole projection is x32-scaled (fp8 cat weights in e4m3 normal
            # range); the drain descales by 1/32.
            def finish_outproj(mts, c=c, cat8=cat8, p_t=p_t):
                t0f = c * CH
                bn3 = bounce_in[c][:].rearrange("(a p) t -> p a t", p=128)
                for mt in mts:
                    mm = psmm.tile([128, CH], F32, tag="mm",
                                   name=f"opc{c}_{mt}")
                    for two in range(2):
                        nc.tensor.matmul(
                            mm, wo8_sb[:, two, mt * 128:(mt + 1) * 128],
                            cat8[:, two, :],
                            start=(two == 0), stop=False,
                            skip_group_check=True)
                    for kt in range(KO - 2):
                        nc.tensor.matmul(
                            mm,
                            wo_sb[:, kt, mt * 128:(mt + 1) * 128],
                            p_t[kt],
                            start=False, stop=(kt == KO - 3),
                            skip_group_check=True)
                    st = stp.tile([128, CH], F32, tag="st", name=f"stc{c}_{mt}")
                    with (tc.high_priority() if "st" in boosts else nullcontext()):
                        if mt % 2 == 0:
                            nc.scalar.activation(out=st, in_=mm,
                                                 func=AF.Identity,
                                                 scale=1.0 / 32)
                        else:
                            nc.vector.tensor_scalar(out=st, in0=mm,
                                                    scalar1=1.0 / 32,
                                                    scalar2=None, op0=OP.mult)
                    nc.sync.dma_start(out=bn3[:, mt, :], in_=st)
                if mts[-1] != MO - 1:
                    return
                if not skip_collective:
                    nc.gpsimd.collective_compute(
                        "ReduceScatter", OP.add,
                        ins=[bounce_in[c][:].opt()],
                        outs=[bounce_out[c][:].opt()],
                        replica_groups=RG)
                    nc.sync.dma_start(out=out_sh[:, t0f:t0f + CH],
                                      in_=bounce_out[c][:])
            pending[c] = finish_outproj

        pending.pop(NCH - 1)(list(range(MO)))

        if debug_partial:
            for c in range(NCH):
                nc.sync.dma_start(out=partial_dbg[:, c * CH:(c + 1) * CH],
                                  in_=bounce_in[c][:])
    nc.compile()
    return nc


# ---------------- host-side sharding ----------------

def _rope_tables():
    inv = 1.0 / (10000.0 ** (np.arange(0, D, 2, dtype=np.float64) / D))
    t = np.arange(L, dtype=np.float64)
    f = t[:, None] * inv[None, :]                 # [L, 32]
    emb = np.concatenate([f, f], axis=1)          # [L, 64]
    cos64 = np.cos(emb).T                         # [64, L]
    sin64 = np.sin(emb).T
    s32 = sin64[0:32]
    sin_signed = np.concatenate([-s32, s32], axis=0)   # [64, L]
    cos2 = np.concatenate([cos64, cos64], axis=0)
    sin2 = np.concatenate([sin_signed, sin_signed], axis=0)
    bf = ml_dtypes.bfloat16
    return cos2.astype(bf), sin2.astype(bf)


def prep_inputs(x, ln_w, ln_b, w_in, w_out, b_out):
    x = np.asarray(x, np.float32)
    ln_w = np.asarray(ln_w, np.float32)
    ln_b = np.asarray(ln_b, np.float32)
    w_in = np.asarray(w_in, np.float32)
    w_out = np.asarray(w_out, np.float32)
    b_out = np.asarray(b_out, np.float32)

    cos2, sin2 = _rope_tables()
    tri = (np.arange(128)[None, :] >= np.arange(128)[:, None]).astype(ml_dtypes.bfloat16)
    lnw0 = ln_w[0:128, None].astype(np.float32)
    lnb0 = ln_b[0:128, None].astype(np.float32)

    xt_b = [np.ascontiguousarray(x[b].T).astype(ml_dtypes.bfloat16) for b in range(B)]

    in_maps = []
    for c in range(N_CORES):
        b, tpi = divmod(c, TP)
        heads = [HPC * tpi + j for j in range(HPC)]
        # qk shard row order: q0,q1, k0,k1, q2,q3, k2,k3, then p(1024)
        rows = []
        for pair in range(HPC // 2):
            h0, h1 = heads[2 * pair], heads[2 * pair + 1]
            rows += list(range(64 * h0, 64 * h0 + 64))
            rows += list(range(64 * h1, 64 * h1 + 64))
            rows += list(range(HID + 64 * h0, HID + 64 * h0 + 64))
            rows += list(range(HID + 64 * h1, HID + 64 * h1 + 64))
        rows += list(range(3 * HID + 1024 * tpi, 3 * HID + 1024 * (tpi + 1)))
        rows = np.array(rows)
        W_sh = w_in[rows, :]                                    # [1536, 1024]
        w_eff = W_sh * ln_w[None, :]
        c1 = W_sh[:, ACC:] @ ln_b[ACC:]
        assert np.abs(c1).max() < 1e-6, (
            "nonzero ln_b[128:] not supported by this build (c1 term dropped)")
        wqf = np.zeros((KQ * 128, MTOT * 128), np.float32)
        wqf[0:896] = w_eff[:, ACC:].T                           # channels 128..1023
        wqf[896:1024] = 0.2 * W_sh[:, 0:ACC].T
        qs = 1.0 / float(D) ** 0.5                # fold q * D^-1/2 into weights
        wqf[:, 0:128] *= qs
        wqf[:, 256:384] *= qs
        # qk m-tile columns x32 so the fp8 copies sit in e4m3's normal range;
        # the kernel descales by 1/32 when reading the PSUM accumulator
        wq8 = (wqf[:, 0:MQK * 128] * 32.0).astype(ml_dtypes.float8_e4m3)
        wq = wqf[:, MQK * 128:]                                 # p rows, bf16
        # v weights: [hid k-tiles, (head, d)] columns
        vrows = []
        for h in heads:
            vrows += list(range(2 * HID + 64 * h, 2 * HID + 64 * h + 64))
        Wv = w_in[np.array(vrows), :]                           # [256, 1024]
        wv_eff = Wv * ln_w[None, :]
        wv = np.zeros((KQ * 128, HPC * 64), np.float32)
        wv[0:896] = wv_eff[:, ACC:].T
        wv[896:1024] = 0.2 * Wv[:, 0:ACC].T
        # out-proj shard: columns [256*tpi:256*(tpi+1)] (o) + [1024+1024*tpi ...]
        # (p), all x32-scaled (descaled in the drain); o-rows also in fp8
        ocols = list(range(256 * tpi, 256 * (tpi + 1)))
        pcols = list(range(HID + 1024 * tpi, HID + 1024 * (tpi + 1)))
        wof = w_out[:, ocols + pcols].T * 32.0                  # [1280, 1024]
        wo8 = np.ascontiguousarray(wof[0:256]).astype(ml_dtypes.bfloat16)
        wo = wof[256:]                                          # p rows, bf16
        in_maps.append({
            "xt": xt_b[b],
            "wq": np.ascontiguousarray(wq).astype(ml_dtypes.bfloat16),
            "wq8": wq8,
            "wv": wv.astype(ml_dtypes.bfloat16),
            "wo": np.ascontiguousarray(wo).astype(ml_dtypes.bfloat16),
            "wo8": wo8,
            "cos2": cos2, "sin2": sin2, "tri": tri,
            "lnw0": lnw0, "lnb0": lnb0,
        })
    return in_maps


def assemble(results, b_out):
    """results: list of 8 per-core dicts with 'out_sh' [256, L] f32.
    b_out is added host-side (the device computes the bias-free GEMM)."""
    out = np.empty((B, L, HID), np.float32)
    for c in range(N_CORES):
        b, tpi = divmod(c, TP)
        out[b, :, 256 * tpi:256 * (tpi + 1)] = results[c]["out_sh"].T
    if np.any(b_out):
        out += np.asarray(b_out, np.float32)[None, None, :]
    return out


_NC_CACHE = {}


def _get_nc():
    if "nc" not in _NC_CACHE:
        _NC_CACHE["nc"] = build_nc()
    return _NC_CACHE["nc"]


def kernel(x, ln_w, ln_b, w_in, w_out, b_out):
    from concourse.bass_utils import run_bass_kernel_spmd
    in_maps = prep_inputs(x, ln_w, ln_b, w_in, w_out, b_out)
    nc = _get_nc()
    res = run_bass_kernel_spmd(nc, in_maps, core_ids=list(range(N_CORES)))
    return assemble(res.results, b_out)
